# revision 19
# baseline (speedup 1.0000x reference)
"""TRN2 Bass kernel for nn_BalancedHamiltonLayer.

Math: out[n,k,j] = sum_{r,s,i} x[n,s,i] * factors_B[r,j,i] * H(A)[r,k,s] + bias
collapses to a single dense matmul  out = x2d @ W + bias  with
W[(s,i),(k,j)] = sum_r H[r,k,s] * B[r,j,i]  (a 1024x1024 matrix folded on host
in float64).

Sharding: data-parallel over the 8192 token rows across 8 NeuronCores
(1024 rows each); W replicated.  The matmul runs in fp16 on the PE
(full-rate, ~5e-4 relative error; fp32 PSUM accumulation).  x is passed
pre-transposed per core as [m2, f_in, 256 tokens] so lhsT tiles load
contiguously (512B bursts).

Schedule (from NTFF trace analysis; baseline 50.1us -> ~43.3us):
- Gate loads (w chunk 0 + first half of x slab 0) issue in parallel on
  the sync and scalar HWDGE queues; remaining loads follow on sync in
  consumption-deadline order.  Slabs 0/1 are k-split so phase 1 gates
  on 0.75 MiB, not 1.25 MiB.  DMA completions smear (descriptors
  round-robin across outstanding transfers), so issue order == deadline
  order matters more than raw sequencing.
- Phase 1 k-interleaves m=0..3 (consumption 1.73us per W chunk) so the
  ~1us/chunk smeared W supply stays ahead; phase 2 is k-contiguous per
  m-tile.
- 8 heavy 512-col warmup matmuls (into a PSUM bank the first real
  start=True matmul resets) run from the earliest tensor-queue slot:
  the HAM controller grants the 2.4GHz window ~4.5-5.5us after heavy
  activity begins, so by the time real data lands the stream runs at
  full clock (light 64-col warmups do NOT trigger the grant).
- Output is stored as fp16 (halves store traffic and PSUM-evict time);
  host converts to fp32 and adds bias.
- The last tile runs n-half-major with its second half in two 256-col
  accumulation groups in separate banks, so the post-matmul tail is
  one narrow CAST + one 128KB store (the ~7us framework teardown that
  follows the final DMA-drain barrier is fixed and unavoidable).
"""

import numpy as np
import concourse.bacc as bacc
import concourse.mybir as mybir
import concourse.tile as tile
from concourse.bass_utils import run_bass_kernel_spmd

B, T, D = 4, 2048, 1024
RANK, FACTOR, SUB = 8, 64, 4
S = 4 * SUB  # 16
NCORES = 8
NTOK = B * T // NCORES  # 1024 token rows per core
P = 128
KT = D // P     # 8 contraction chunks
MT = NTOK // P  # 8 token tiles per core
M2 = MT // 2    # x DMA granularity: 256-token slabs
NH = 512        # f_out half (one PSUM bank)
NWARM = 8       # PE warmup matmuls (512 cols each, heavy -> HAM boost grant)
NP1 = 4         # phase-1 m-tiles (k-interleaved; consumption 1.73us/chunk
                # stays above the ~1us/chunk smeared W supply rate)

_cached_nc = None


def build_module():
    global _cached_nc
    if _cached_nc is not None:
        return _cached_nc
    nc = bacc.Bacc("TRN2", target_bir_lowering=False, debug=False)
    xT = nc.dram_tensor("xT", [M2, D, 2 * P], mybir.dt.float16, kind="ExternalInput").ap()
    w = nc.dram_tensor("w", [D, D], mybir.dt.float16, kind="ExternalInput").ap()
    out = nc.dram_tensor("out", [NTOK, D], mybir.dt.float16, kind="ExternalOutput").ap()

    with tile.TileContext(nc) as tc:
        with (
            tc.tile_pool(name="wp", bufs=1) as wp,
            tc.tile_pool(name="xp", bufs=1) as xp,
            tc.tile_pool(name="op", bufs=4) as op,
            tc.tile_pool(name="ps", bufs=4, space="PSUM") as ps,
        ):
            # Warmup operand: memset on gpsimd (its preamble ends
            # earliest, so warmup matmuls start sooner and the HAM boost
            # grant arrives before real data).  Warmup matmuls write into
            # pts[0][0]'s bank; the first real start=True matmul resets
            # it, so their values are irrelevant.
            g = xp.tile([P, NH], mybir.dt.float16, tag="warm", name="g")
            nc.gpsimd.memset(g[:], 0.0)

            xt = {}
            wt = {}

            def x_tile(m2, name):
                t = xp.tile([P, KT, 2 * P], mybir.dt.float16, tag=f"x{m2}", name=name)
                xt[m2] = t
                return t

            def w_tile(k, name):
                t = wp.tile([P, 2 * NH], mybir.dt.float16, tag=f"w{k}", name=name)
                wt[k] = t
                return t

            for m2 in range(M2):
                x_tile(m2, f"xt{m2}")
            for k in range(KT):
                w_tile(k, f"wt{k}")

            def xsrc(m2):
                return xT[m2].rearrange("(k p) t -> p k t", p=P)

            # Single ring, strict deadline order: sequential draining means
            # early slots COMPLETE early (parallel rings share SDMA packet
            # round-robin, which makes every transfer finish late together).
            # Slabs 0/1 are split at k=4 so the first matmul group (k=0)
            # gates on w0 + two 256KB half-slabs, not 1 MiB.  DMA
            # completions smear (descriptors round-robin across
            # outstanding transfers), so order = consumption deadline.
            kh = KT // 2

            def xhalf(m2, h):
                return (
                    xt[m2][:, h * kh:(h + 1) * kh, :],
                    xsrc(m2)[:, h * kh:(h + 1) * kh, :],
                )

            # xslab0's first half is issued from the scalar queue in
            # parallel with w0 on the sync queue — the two 256KB gate
            # transfers start together instead of serializing behind a
            # second ~700ns issue slot.
            da, sa = xhalf(0, 0)
            nc.scalar.dma_start(da, sa)
            loads = [
                (wt[0][:], w[0:P, :]),
                xhalf(1, 0),
                (wt[1][:], w[P:2 * P, :]),
                (wt[2][:], w[2 * P:3 * P, :]),
                (wt[3][:], w[3 * P:4 * P, :]),
                xhalf(0, 1),
                xhalf(1, 1),
            ] + [
                (wt[k][:], w[k * P:(k + 1) * P, :]) for k in range(4, KT)
            ] + [
                (xt[2][:], xsrc(2)),
                (xt[3][:], xsrc(3)),
            ]
            for da, sa in loads:
                nc.sync.dma_start(da, sa)

            def emit_half(m, pt, n):
                o = op.tile([P, NH], mybir.dt.float16, tag="o", name="o")
                nc.vector.tensor_copy(o[:], pt[n][:])
                nc.sync.dma_start(
                    out[m * P:(m + 1) * P, n * NH:(n + 1) * NH], o[:]
                )

            def emit_out(m, pt):
                for n in range(2):
                    emit_half(m, pt, n)

            def xs_of(m):
                return xt[m // 2][:, :, (m % 2) * P:(m % 2 + 1) * P]

            with nc.named_scope("mm"):
                pts = {
                    m: {
                        n: ps.tile([P, NH], mybir.dt.float32, tag=f"ps{n}", name=f"pt{m}_{n}")
                        for n in range(2)
                    }
                    for m in range(NP1)
                }
                # PE warmup: heavy 512-col matmuls into pts[0][0]'s bank
                # (discarded by the real start=True matmul below).  Heavy
                # sustained activity from ~6.4us triggers the HAM
                # full-clock grant (~3.2us of high activity) BEFORE real
                # data lands, so the real stream runs at 2.4GHz from the
                # first matmul.  Light 64-col warmups do NOT trigger the
                # grant (measured: 4.4us of half-clock real matmuls).
                for i in range(NWARM):
                    nc.tensor.matmul(
                        pts[0][0][:], g[:, :P], g[:],
                        start=True, stop=True, skip_group_check=True,
                    )
                # Phase 1: m=0,1 k-interleaved — gates only on w0+xslab0;
                # per-k compute (4 MMs, ~0.86us) exceeds the W-chunk
                # arrival cadence (~0.73us), so the PE absorbs
                # DMA-completion jitter without stalling.
                for k in range(KT):
                    for m in range(NP1):
                        for n in range(2):
                            nc.tensor.matmul(
                                pts[m][n][:],
                                xs_of(m)[:, k, :],
                                wt[k][:, n * NH:(n + 1) * NH],
                                start=(k == 0),
                                stop=(k == KT - 1),
                            )
                for m in range(NP1):
                    emit_out(m, pts[m])

                # Phase 2: k-contiguous per m-tile (PE stays warm, dense).
                # The last tile runs n-half-major so its n=0 half is
                # emitted while n=1 still computes — the post-matmul tail
                # is then one CAST + one store instead of the whole tile.
                for m in range(NP1, MT):
                    nps = 1 if m == MT - 1 else 2
                    pt = {
                        n: ps.tile([P, NH], mybir.dt.float32, tag=f"ps{n}", name=f"pt{n}")
                        for n in range(nps)
                    }
                    if m == MT - 1:
                        # n=0 full-width, then n=1 in two 256-col
                        # accumulation groups (separate banks, so the
                        # second group's matmuls don't WAR-stall on the
                        # first group's CAST) — the post-matmul tail is
                        # one narrow CAST + one 128KB store.
                        for k in range(KT):
                            nc.tensor.matmul(
                                pt[0][:],
                                xs_of(m)[:, k, :],
                                wt[k][:, 0:NH],
                                start=(k == 0),
                                stop=(k == KT - 1),
                            )
                        emit_half(m, pt, 0)
                        for q in range(2):
                            c0 = NH + q * (NH // 2)
                            ptq = ps.tile(
                                [P, NH], mybir.dt.float32,
                                tag="ps1", name=f"ptq{q}",
                            )
                            for k in range(KT):
                                nc.tensor.matmul(
                                    ptq[:, 0:NH // 2],
                                    xs_of(m)[:, k, :],
                                    wt[k][:, c0:c0 + NH // 2],
                                    start=(k == 0),
                                    stop=(k == KT - 1),
                                )
                            o = op.tile([P, NH // 2], mybir.dt.float16, tag="oq", name="oq")
                            nc.vector.tensor_copy(o[:], ptq[:, 0:NH // 2])
                            nc.sync.dma_start(
                                out[m * P:(m + 1) * P, c0:c0 + NH // 2], o[:]
                            )
                    else:
                        for k in range(KT):
                            for n in range(2):
                                nc.tensor.matmul(
                                    pt[n][:],
                                    xs_of(m)[:, k, :],
                                    wt[k][:, n * NH:(n + 1) * NH],
                                    start=(k == 0),
                                    stop=(k == KT - 1),
                                )
                        emit_out(m, pt)
    nc.compile()
    _cached_nc = nc
    return nc


def _construct_hamilton(A):
    # A: [rank, 4, sub, sub] -> [rank, 4*sub, 4*sub]
    r, i, j, k = A[:, 0], A[:, 1], A[:, 2], A[:, 3]
    return np.concatenate(
        [
            np.concatenate([r, -i, -j, -k], axis=2),
            np.concatenate([i, r, -k, j], axis=2),
            np.concatenate([j, k, r, -i], axis=2),
            np.concatenate([k, -j, i, r], axis=2),
        ],
        axis=1,
    )


def build_in_maps(x, A, factors_B):
    H = _construct_hamilton(np.asarray(A, dtype=np.float64))  # [r, k, s]
    Bf = np.asarray(factors_B, dtype=np.float64)  # [r, j, i]
    # W[(s,i),(k,j)] = sum_r H[r,k,s] * B[r,j,i]
    W = np.einsum("rks,rji->sikj", H, Bf).reshape(D, D).astype(np.float16)

    x2 = np.asarray(x, dtype=np.float16).reshape(NCORES, NTOK, D)
    in_maps = []
    for c in range(NCORES):
        # [NTOK, D] -> [M2, 256, D] -> [M2, D, 256]
        xs = np.ascontiguousarray(x2[c].reshape(M2, 2 * P, D).transpose(0, 2, 1))
        in_maps.append({"xT": xs, "w": W})
    return in_maps


def kernel(x, A, factors_B, bias):
    nc = build_module()
    in_maps = build_in_maps(x, A, factors_B)
    br = run_bass_kernel_spmd(nc, in_maps, core_ids=list(range(NCORES)))
    out = np.concatenate([r["out"] for r in br.results], axis=0)
    out = out.astype(np.float32) + np.asarray(bias, dtype=np.float32)[None, :]
    return out.reshape(B, T, D)
